# revision 1
# baseline (speedup 1.0000x reference)
"""Biased multi-head attention block (LayerNorm -> QKV -> attn+bias softmax -> out proj)
on 8 Trainium2 NeuronCores, data-parallel over the batch dimension (one batch element
per core).

Per-core device kernel layout strategy (all matmul operands bf16):
  - All PE operands are bf16: the xbus streams 2 bytes/cycle/partition, so a
    512-wide bf16 moving tile takes ~213ns vs ~426+ for 4-byte float32r
    ("full-rate" f32r refers to MAC rate, not streaming). Weight DMA halves.
  - LayerNorm in [token, dim] (bn_stats/bn_aggr + fused tensor_scalar-with-
    bf16-cast), then PE-transpose of the bf16 tiles to xnT [dim, token];
    transpose evictions alternate vector/scalar engines. (A DMA-XBAR
    dma_start_transpose variant was ~27 GB/s/queue AND produced
    nondeterministic corruption on hardware — do not revisit.)
  - V lands in [token, feat] (xnT stationary, wv moving) with an all-ones
    column per head so the softmax denominators fall out of the attn @ V
    matmul; V PSUM evictions run on the scalar engine (idle in this phase).
  - Q,K are projected into qT/kT [feat, token]; each head pair's projection is
    slotted 4-matmuls-per-step into the PREVIOUS pair's attention steps.
  - Attention per pair, per j-tile: the two heads' simT[j,i] = k_h^T q_h go to
    two separate [128, 1024] fp32 PSUM tiles; the per-hf matmul pairs target
    disjoint PE row groups (head0 rows 0-63 @ tile (0,0), head1 rows 64-127 @
    (64,0)) so they execute concurrently. exp() on the scalar engine with the
    1/8 scale folded in; the additive attention bias is a vector-engine
    multiply by host-precomputed exp(bias)^T in bf16 (exp(a+b)=exp(a)exp(b)).
  - PSUM budget (8 banks): 2 sim tiles (4) + head0 av accumulator (2) +
    projection accumulator (2). Head1's AV matmuls are deferred: its et tiles
    are retained in SBUF and the 16-matmul block runs rotated into the NEXT
    pair's stream (overlapping its first exps); the last pair, which has no
    projection work, runs head1's av inline in the projection bank instead.
  - Softmax denominators: reciprocal as exp(-ln(sums)) on the scalar engine
    (a [16,1024] bf16 DVE reciprocal measures ~8us). Rows 0-13 are computed
    early inside the last pair, and the first three per-pair normalizations
    (selection-matrix broadcast matmul + multiply) are slotted into the last
    pair's attention steps, so the tail has ready work immediately and the PE
    never idles into a HAM re-throttle (which would halve the PE clock for
    the whole output projection).
  - Output projection y = outT^T @ w_out runs kt-major in 2-token-tile chunks
    so accumulation starts as soon as the early outT tiles are normalized.

Measured on hardware: ~332us exec per core (8 cores in parallel),
rel err ~6.8e-3 vs the fp32 reference (baseline at session start: 421-435us
at 3.0e-3; tolerance 2e-2).
"""

import os

import numpy as np
import ml_dtypes

import concourse.bacc as bacc
import concourse.bass as bass
import concourse.mybir as mybir
import concourse.tile as tile
from concourse.bass_utils import run_bass_kernel_spmd
from concourse.masks import make_identity

B = 8
N = 1024
DIM = 1024
HEADS = 16
DH = 64
INNER = HEADS * DH
P = 128
NT = N // P          # token tiles
KT = DIM // P        # contraction tiles
PAIRS = HEADS // 2   # head pairs (one qT/kT feature tile each)
EPS = 1e-5
SCALE = DH ** -0.5   # 0.125, exact in fp32

F32 = mybir.dt.float32
BF16 = mybir.dt.bfloat16
AF = mybir.ActivationFunctionType

_BUILD_CACHE = {}


def _build(apply_gamma: bool, apply_beta: bool):
    key = (apply_gamma, apply_beta)
    if key in _BUILD_CACHE:
        return _BUILD_CACHE[key]

    nc = bacc.Bacc("TRN2", target_bir_lowering=False, debug=False)

    x_d = nc.dram_tensor("x", [N, DIM], F32, kind="ExternalInput")
    wqk_d = nc.dram_tensor("wqk", [PAIRS, P, KT, 2 * P], BF16, kind="ExternalInput")
    wv_d = nc.dram_tensor("wv", [KT, P, DIM], BF16, kind="ExternalInput")
    wo_d = nc.dram_tensor("wo", [P, KT, DIM], BF16, kind="ExternalInput")
    # exp(bias)^T per head pair, [pair, jt] -> [P(j), 2N] (head0 | head1)
    bias_d = nc.dram_tensor("biasT", [PAIRS, NT, P, 2 * N], BF16, kind="ExternalInput")
    s2_d = nc.dram_tensor("s2", [2, P], BF16, kind="ExternalInput")
    sel_d = nc.dram_tensor("sel", [HEADS, KT * P], BF16, kind="ExternalInput")
    gamma_d = beta_d = None
    if apply_gamma:
        gamma_d = nc.dram_tensor("gamma", [DIM], F32, kind="ExternalInput")
    if apply_beta:
        beta_d = nc.dram_tensor("beta", [DIM], F32, kind="ExternalInput")
    y_d = nc.dram_tensor("y", [N, DIM], F32, kind="ExternalOutput")

    with tile.TileContext(nc) as tc:
        from contextlib import ExitStack

        with ExitStack() as ctx:
            consts = ctx.enter_context(tc.tile_pool(name="consts", bufs=1))
            xpool = ctx.enter_context(tc.tile_pool(name="xpool", bufs=3))
            xbpool = ctx.enter_context(tc.tile_pool(name="xbpool", bufs=3))
            stats = ctx.enter_context(tc.tile_pool(name="stats", bufs=4))
            bigp = ctx.enter_context(tc.tile_pool(name="bigp", bufs=1))
            vpool = ctx.enter_context(tc.tile_pool(name="vpool", bufs=NT))
            wstream = ctx.enter_context(tc.tile_pool(name="wstream", bufs=3))
            qkpool = ctx.enter_context(tc.tile_pool(name="qkpool", bufs=4))
            e0pool = ctx.enter_context(tc.tile_pool(name="e0pool", bufs=3))
            e1pool = ctx.enter_context(tc.tile_pool(name="e1pool", bufs=NT + 2))
            bpool = ctx.enter_context(tc.tile_pool(name="bpool", bufs=3))
            opool = ctx.enter_context(tc.tile_pool(name="opool", bufs=KT))
            wvpool = ctx.enter_context(tc.tile_pool(name="wvpool", bufs=KT))

            eps_t = consts.tile([P, 1], F32, name="eps_t")
            nc.vector.memset(eps_t, EPS)
            ident_bf = consts.tile([P, P], BF16, name="ident_bf")
            make_identity(nc, ident_bf)
            # Per-pair denominator broadcast: S2[0, 0:64]=1 (head0 rows),
            # S2[1, 64:128]=1 (head1 rows).
            S2 = consts.tile([2, P], BF16, name="S2")
            nc.sync.dma_start(out=S2, in_=s2_d[:, :])
            S = consts.tile([HEADS, KT * P], BF16, name="S")
            nc.sync.dma_start(out=S, in_=sel_d[:, :])
            sums = consts.tile([HEADS, N], BF16, name="sums")
            recip = consts.tile([HEADS, N], BF16, name="recip")
            lsum = consts.tile([HEADS, N], F32, name="lsum")
            nc.vector.memset(recip, 1.0)

            gamma_t = beta_t = None
            if apply_gamma:
                gamma_t = consts.tile([P, DIM], F32, name="gamma_t")
                g_ap = gamma_d[:]
                nc.sync.dma_start(
                    out=gamma_t,
                    in_=bass.AP(
                        tensor=g_ap.tensor, offset=g_ap.offset, ap=[[0, P]] + list(g_ap.ap)
                    ),
                )
            if apply_beta:
                beta_t = consts.tile([P, DIM], F32, name="beta_t")
                b_ap = beta_d[:]
                nc.sync.dma_start(
                    out=beta_t,
                    in_=bass.AP(
                        tensor=b_ap.tensor, offset=b_ap.offset, ap=[[0, P]] + list(b_ap.ap)
                    ),
                )

            xnT = bigp.tile([P, KT, N], BF16, name="xnT", tag="big")

            vts = []
            for jt in range(NT):
                vt = vpool.tile([P, HEADS * (DH + 1)], BF16, name=f"v{jt}", tag="v")
                vv = vt.rearrange("p (h c) -> p h c", c=DH + 1)
                nc.vector.memset(vv[:, :, DH : DH + 1], 1.0)
                vts.append((vt, vv))

            # ================= Phase A: LayerNorm + DMA transpose + V =======
            def emit_ln(it):
                xt = xpool.tile([P, DIM], F32, name=f"x{it}", tag="x")
                nc.sync.dma_start(out=xt, in_=x_d[it * P : (it + 1) * P, :])
                st = stats.tile([P, 2, 6], F32, name=f"st{it}", tag="st")
                nc.vector.bn_stats(out=st[:, 0], in_=xt[:, 0:512])
                nc.vector.bn_stats(out=st[:, 1], in_=xt[:, 512:1024])
                mv = stats.tile([P, 2], F32, name=f"mv{it}", tag="mv")
                nc.vector.bn_aggr(out=mv, in_=st)
                std = stats.tile([P, 1], F32, name=f"sd{it}", tag="sd")
                nc.scalar.activation(out=std, in_=mv[:, 1:2], func=AF.Sqrt, bias=eps_t)
                rstd = stats.tile([P, 1], F32, name=f"rs{it}", tag="rs")
                nc.vector.reciprocal(out=rstd, in_=std)
                xb = xbpool.tile([P, DIM], BF16, name=f"xb{it}", tag="xb")
                if gamma_t is None and beta_t is None:
                    nc.vector.tensor_scalar(
                        out=xb,
                        in0=xt,
                        scalar1=mv[:, 0:1],
                        scalar2=rstd,
                        op0=mybir.AluOpType.subtract,
                        op1=mybir.AluOpType.mult,
                    )
                else:
                    nc.vector.tensor_scalar(
                        out=xt,
                        in0=xt,
                        scalar1=mv[:, 0:1],
                        scalar2=rstd,
                        op0=mybir.AluOpType.subtract,
                        op1=mybir.AluOpType.mult,
                    )
                    if gamma_t is not None:
                        nc.vector.tensor_mul(xt, xt, gamma_t)
                    if beta_t is not None:
                        nc.vector.tensor_add(xt, xt, beta_t)
                    nc.vector.tensor_copy(xb, xt)
                # PE transpose each [128,128] bf16 block into xnT (the
                # DMA-XBAR transpose corrupted results on hardware — its
                # completion sync raced the consuming matmuls — and at
                # ~27 GB/s/queue it was also slower than the PE).
                for kt in range(KT):
                    pt = ptpool.tile([P, P], BF16, name=f"tp{it}_{kt}", tag="tp")
                    nc.tensor.transpose(
                        pt, xb[:, kt * P : (kt + 1) * P], ident_bf
                    )
                    if kt % 2 == 0:
                        nc.vector.tensor_copy(
                            xnT[:, kt, it * P : (it + 1) * P], pt
                        )
                    else:
                        nc.scalar.copy(
                            out=xnT[:, kt, it * P : (it + 1) * P], in_=pt
                        )

            wvts = []
            with tc.tile_pool(name="psA", bufs=2, space="PSUM") as psA, \
                 tc.tile_pool(name="ptp", bufs=4, space="PSUM") as ptpool:

                def emit_v_group(g):
                    psv = [
                        psA.tile([P, DIM], F32, name=f"psv{g}_{j}", tag="psv")
                        for j in range(2)
                    ]
                    for kt in range(KT):
                        if g == 0:
                            wvt = wvpool.tile(
                                [P, DIM], BF16, name=f"wv{kt}", tag="wv"
                            )
                            nc.sync.dma_start(out=wvt, in_=wv_d[kt])
                            wvts.append(wvt)
                        wvt = wvts[kt]
                        for j in range(2):
                            jt = 2 * g + j
                            for hf in range(2):
                                sl = slice(hf * 512, hf * 512 + 512)
                                nc.tensor.matmul(
                                    psv[j][:, sl],
                                    lhsT=xnT[:, kt, jt * P : (jt + 1) * P],
                                    rhs=wvt[:, sl],
                                    start=(kt == 0),
                                    stop=(kt == KT - 1),
                                )
                    # Evictions on the scalar engine (idle in this phase).
                    for j in range(2):
                        jt = 2 * g + j
                        vv = vts[jt][1]
                        for hf in range(2):
                            nc.scalar.copy(
                                out=vv[:, hf * 8 : hf * 8 + 8, 0:DH],
                                in_=psv[j][
                                    :, hf * 512 : hf * 512 + 512
                                ].rearrange("p (h c) -> p h c", c=DH),
                            )

                for g in range(NT // 2):
                    emit_ln(2 * g)
                    emit_ln(2 * g + 1)
                    emit_v_group(g)

            # ================= Phase B: QK proj + attention =================
            wqs, qTs, kTs = [], [], []
            wq0 = wstream.tile([P, KT, 2 * P], BF16, name="wqk0", tag="w")
            nc.sync.dma_start(out=wq0, in_=wqk_d[0])
            wqs.append(wq0)
            qTs.append(qkpool.tile([P, N], BF16, name="qT0", tag="qk"))
            kTs.append(qkpool.tile([P, N], BF16, name="kT0", tag="qk"))

            with ExitStack() as actx:
                sim_pool = actx.enter_context(
                    tc.tile_pool(name="simp", bufs=2, space="PSUM")
                )
                av_pool = actx.enter_context(
                    tc.tile_pool(name="avp", bufs=1, space="PSUM")
                )
                proj_pool = actx.enter_context(
                    tc.tile_pool(name="projp", bufs=1, space="PSUM")
                )

                def proj_steps(pn, which):
                    """Generator: 16 matmuls (kt-outer, hf-inner) accumulating
                    pair pn's q (which=0) or k (which=1) projection, then
                    evicts to SBUF. Yields after each matmul."""
                    ps = proj_pool.tile(
                        [P, N], F32, name=f"ps{'qk'[which]}{pn}", tag="proj"
                    )
                    w0 = which * P
                    for kt in range(KT):
                        for hf in range(2):
                            sl = slice(hf * 512, hf * 512 + 512)
                            nc.tensor.matmul(
                                ps[:, sl],
                                lhsT=wqs[pn][:, kt, w0 : w0 + P],
                                rhs=xnT[:, kt, sl],
                                start=(kt == 0),
                                stop=(kt == KT - 1),
                            )
                            yield
                    nc.vector.tensor_copy((qTs, kTs)[which][pn], ps)
                    while True:
                        yield

                # Pair 0's projections run standalone (prologue).
                for which in range(2):
                    g = proj_steps(0, which)
                    for _ in range(17):
                        next(g)

                outTs = []

                def emit_av(avt, pp, ets_, hh, jt):
                    h = 2 * pp + hh
                    for hf in range(2):
                        sl = slice(hf * 512, hf * 512 + 512)
                        nc.tensor.matmul(
                            avt[:, sl],
                            lhsT=vts[jt][0][:, h * (DH + 1) : (h + 1) * (DH + 1)],
                            rhs=ets_[jt][hh][:, sl],
                            start=(jt == 0),
                            stop=(jt == NT - 1),
                        )

                def emit_evict(avt, pp, hh):
                    h = 2 * pp + hh
                    hs = slice(hh * DH, (hh + 1) * DH)
                    # Evict via SBUF staging (DMA cannot read PSUM; DVE
                    # cannot shift partitions — stage on matching partitions,
                    # then DMA to the head's row block in outT and its row of
                    # the pair's denominator staging tile).
                    avs = xpool.tile([DH + 1, N], BF16, name=f"avs{h}", tag="avs")
                    nc.vector.tensor_copy(avs, avt)
                    nc.sync.dma_start(out=outTs[pp][hs, :], in_=avs[0:DH, :])
                    h_ = 2 * pp + hh
                    nc.sync.dma_start(out=sums[h_ : h_ + 1, :], in_=avs[DH : DH + 1, :])

                def emit_norm_kt(kt, pool):
                    # Normalize pair kt's outT rows: broadcast recip rows
                    # [2kt, 2kt+2) over the 128 feature rows (S columns for
                    # other heads are zero) and multiply.
                    rs = pool.tile([P, N], F32, name=f"rs{kt}", tag="sim")
                    for hf in range(2):
                        sl = slice(hf * 512, hf * 512 + 512)
                        nc.tensor.matmul(
                            rs[:, sl],
                            lhsT=S[:, kt * P : (kt + 1) * P],
                            rhs=recip[:, sl],
                            start=True,
                            stop=True,
                        )
                    nc.vector.tensor_mul(outTs[kt], outTs[kt], rs)

                def emit_av1_block(pp, ets_):
                    # Head1's AV for pair pp, run from the retained et tiles
                    # (the av bank is free after head0's eviction).
                    av1 = av_pool.tile([DH + 1, N], F32, name=f"av{2*pp+1}", tag="av")
                    for jt in range(NT):
                        emit_av(av1, pp, ets_, 1, jt)
                    emit_evict(av1, pp, 1)

                prev_ets = None
                for p in range(PAIRS):
                    qT, kTt = qTs[p], kTs[p]
                    if p + 1 < PAIRS:
                        wq = wstream.tile(
                            [P, KT, 2 * P], BF16, name=f"wqk{p+1}", tag="w"
                        )
                        nc.sync.dma_start(out=wq, in_=wqk_d[p + 1])
                        wqs.append(wq)
                        qTs.append(
                            qkpool.tile([P, N], BF16, name=f"qT{p+1}", tag="qk")
                        )
                        kTs.append(
                            qkpool.tile([P, N], BF16, name=f"kT{p+1}", tag="qk")
                        )

                    ot = opool.tile([P, N], BF16, name=f"outT{p}", tag="outT")
                    outTs.append(ot)
                    ets = []
                    av0 = None

                    if p + 1 < PAIRS:
                        projq = proj_steps(p + 1, 0)
                        projk = proj_steps(p + 1, 1)
                    else:
                        projq = projk = iter(())
                    pstep = [0]

                    def proj_step(k):
                        # Steps 0-16 drive the q projection (16 matmuls + its
                        # eviction), 17-33 the k projection.
                        for _ in range(k):
                            if pstep[0] < 17:
                                next(projq, None)
                            elif pstep[0] < 34:
                                next(projk, None)
                            pstep[0] += 1

                    av1l = None  # last pair only: av1 inline in the proj bank
                    for jt in range(NT):
                        bts = []
                        for hh in range(2):
                            bth = bpool.tile(
                                [P, N], BF16, name=f"b{p}_{jt}_{hh}", tag="bias"
                            )
                            nc.sync.dma_start(
                                out=bth, in_=bias_d[p, jt, :, hh * N : (hh + 1) * N]
                            )
                            bts.append(bth)
                        sims = [
                            sim_pool.tile([P, N], F32, name=f"sim{p}_{jt}_{hh}", tag="sim")
                            for hh in range(2)
                        ]
                        # The two heads' sims target disjoint PE row groups
                        # (rows 0-63 / 64-127) and distinct PSUM banks -> each
                        # hf's pair of matmuls runs concurrently.
                        for hf in range(2):
                            sl = slice(hf * 512, hf * 512 + 512)
                            for hh in range(2):
                                hs = slice(hh * DH, (hh + 1) * DH)
                                nc.tensor.matmul(
                                    sims[hh][:, sl],
                                    lhsT=kTt[hs, jt * P : (jt + 1) * P],
                                    rhs=qT[hs, sl],
                                    start=True,
                                    stop=True,
                                )
                        ep = (e0pool, e1pool)
                        ett = []
                        for hh in range(2):
                            et = ep[hh].tile(
                                [P, N], BF16, name=f"e{p}_{jt}_{hh}", tag="exp"
                            )
                            nc.scalar.activation(
                                out=et, in_=sims[hh], func=AF.Exp, scale=SCALE
                            )
                            nc.vector.tensor_mul(et, et, bts[hh])
                            ett.append(et)
                        ets.append(ett)
                        if jt == 1 and prev_ets is not None:
                            # Rotated: the previous pair's head1 AV block runs
                            # here, overlapping this pair's first exps.
                            emit_av1_block(p - 1, prev_ets)
                            if p == PAIRS - 1:
                                # Pairs 0-6 denominators are final: compute
                                # their reciprocals now (rows 0-13) so the
                                # normalization needs no serial chain at the
                                # tail.
                                nc.scalar.activation(
                                    out=lsum[0:14, :], in_=sums[0:14, :], func=AF.Ln
                                )
                                nc.scalar.activation(
                                    out=recip[0:14, :], in_=lsum[0:14, :],
                                    func=AF.Exp, scale=-1.0,
                                )
                        if p == PAIRS - 1 and jt in (3, 5, 7):
                            # Slot the first three normalizations into the
                            # last pair's stream (it has no projection work).
                            emit_norm_kt((jt - 3) // 2, sim_pool)
                        if jt > 0:
                            if av0 is None:
                                av0 = av_pool.tile(
                                    [DH + 1, N], F32, name=f"av{2*p}", tag="av"
                                )
                            emit_av(av0, p, ets, 0, jt - 1)
                            if p == PAIRS - 1:
                                # No projections in the last pair: its av1
                                # accumulates inline in the free proj bank.
                                if av1l is None:
                                    av1l = proj_pool.tile(
                                        [DH + 1, N], F32, name="av15", tag="proj"
                                    )
                                emit_av(av1l, p, ets, 1, jt - 1)
                        proj_step(4)

                    emit_av(av0, p, ets, 0, NT - 1)
                    proj_step(3)  # drain the projection evictions
                    emit_evict(av0, p, 0)
                    if p == PAIRS - 1:
                        emit_av(av1l, p, ets, 1, NT - 1)
                        emit_evict(av1l, p, 1)
                    else:
                        prev_ets = ets

            # ================= Phase C: normalize + y = outT^T @ w_out ======
            wo_t = bigp.tile([P, KT, DIM], BF16, name="wo_t", tag="big")
            for kt in range(KT):
                nc.sync.dma_start(out=wo_t[:, kt, :], in_=wo_d[:, kt, :])

            with ExitStack() as tctx:
                rs_pool = tctx.enter_context(
                    tc.tile_pool(name="rsp", bufs=2, space="PSUM")
                )
                psy_pool = tctx.enter_context(
                    tc.tile_pool(name="psyp", bufs=4, space="PSUM")
                )

                # Finish the reciprocals (last pair, rows 14-15) and the
                # remaining normalizations; kts 0-2 were normalized inside
                # pair 7, so the output projection has ready operands
                # immediately and the PE never idles into a HAM re-throttle.
                nc.scalar.activation(out=lsum, in_=sums, func=AF.Ln)
                nc.scalar.activation(out=recip, in_=lsum, func=AF.Exp, scale=-1.0)
                for kt in range(3, KT):
                    emit_norm_kt(kt, rs_pool)

                # kt-major output projection in chunks of 2 token tiles: the
                # first chunk's kt accumulation starts as soon as outT[kt] is
                # normalized, keeping the PE dense through the tail.
                for c in range(NT // 2):
                    psys = {}
                    for j in range(2):
                        it = 2 * c + j
                        for hf in range(2):
                            psys[(j, hf)] = psy_pool.tile(
                                [P, 512], F32, name=f"psy{it}_{hf}", tag="psy"
                            )
                    for kt in range(KT):
                        for j in range(2):
                            it = 2 * c + j
                            for hf in range(2):
                                nc.tensor.matmul(
                                    psys[(j, hf)],
                                    lhsT=outTs[kt][:, it * P : (it + 1) * P],
                                    rhs=wo_t[:, kt, hf * 512 : hf * 512 + 512],
                                    start=(kt == 0),
                                    stop=(kt == KT - 1),
                                )
                    for j in range(2):
                        it = 2 * c + j
                        yst = xpool.tile([P, DIM], F32, name=f"y{it}", tag="x")
                        nc.vector.tensor_copy(yst[:, 0:512], psys[(j, 0)])
                        nc.scalar.copy(out=yst[:, 512:1024], in_=psys[(j, 1)])
                        nc.sync.dma_start(
                            out=y_d[it * P : (it + 1) * P, :], in_=yst
                        )

    nc.compile()
    _BUILD_CACHE[key] = nc
    return nc


def _host_prep(ln_gamma, ln_beta, w_qkv, w_out, attn_bias):
    """Re-layout weights/bias for the device kernel (pure host-side reshapes)."""
    w_qkv = np.asarray(w_qkv, np.float32)
    w_out = np.asarray(w_out, np.float32)
    attn_bias = np.asarray(attn_bias, np.float32)

    wq_r = w_qkv[:, :INNER].reshape(KT, P, PAIRS, P).transpose(2, 1, 0, 3)
    wk_r = w_qkv[:, INNER : 2 * INNER].reshape(KT, P, PAIRS, P).transpose(2, 1, 0, 3)
    wqk = np.ascontiguousarray(
        np.concatenate([wq_r, wk_r], axis=3).astype(ml_dtypes.bfloat16)
    )
    wv = np.ascontiguousarray(
        w_qkv[:, 2 * INNER :].reshape(KT, P, DIM).astype(ml_dtypes.bfloat16)
    )
    wo = np.ascontiguousarray(
        w_out.reshape(KT, P, DIM).transpose(1, 0, 2).astype(ml_dtypes.bfloat16)
    )
    # exp(bias), transposed per head to [j, i], paired: [pair, jt, P, h0|h1].
    ebT = (
        np.exp(attn_bias[0].astype(np.float64))
        .astype(np.float32)
        .transpose(0, 2, 1)
        .astype(ml_dtypes.bfloat16)
    )  # [HEADS, N(j), N(i)]
    biasT = np.ascontiguousarray(
        ebT.reshape(PAIRS, 2, NT, P, N).transpose(0, 2, 3, 1, 4).reshape(
            PAIRS, NT, P, 2 * N
        )
    )
    s2 = np.zeros((2, P), dtype=ml_dtypes.bfloat16)
    s2[0, 0:DH] = 1.0
    s2[1, DH:P] = 1.0
    sel = np.zeros((HEADS, KT * P), dtype=ml_dtypes.bfloat16)
    for h in range(HEADS):
        c0 = (h // 2) * P + (h % 2) * DH
        sel[h, c0 : c0 + DH] = 1.0
    in_map = {"wqk": wqk, "wv": wv, "wo": wo, "biasT": biasT, "s2": s2, "sel": sel}

    gamma = np.asarray(ln_gamma, np.float32)
    beta = np.asarray(ln_beta, np.float32)
    apply_gamma = not np.all(gamma == 1.0)
    apply_beta = bool(np.any(beta != 0.0))
    if apply_gamma:
        in_map["gamma"] = gamma
    if apply_beta:
        in_map["beta"] = beta
    return in_map, apply_gamma, apply_beta


def kernel(x, ln_gamma, ln_beta, w_qkv, w_out, attn_bias):
    x = np.asarray(x, np.float32)
    in_map, apply_gamma, apply_beta = _host_prep(
        ln_gamma, ln_beta, w_qkv, w_out, attn_bias
    )
    nc = _build(apply_gamma, apply_beta)
    in_maps = [dict(in_map, x=np.ascontiguousarray(x[b])) for b in range(B)]
    res = run_bass_kernel_spmd(
        nc,
        in_maps,
        list(range(B)),
        trace=bool(int(os.environ.get("BA_TRACE", "0"))),
        tmpdir=os.environ.get("BA_TRACE_DIR") or None,
    )
    out = np.stack([res.results[i]["y"] for i in range(B)], axis=0)
    if bool(int(os.environ.get("BA_TRACE", "0"))):
        kernel.last_exec_time_ns = res.exec_time_ns
        kernel.last_mean_exec_time_ns = res.mean_exec_time_ns
    return out



# revision 3
# speedup vs baseline: 1.1786x; 1.1786x over previous
"""Biased multi-head attention block (LayerNorm -> QKV -> attn+bias softmax -> out proj)
on 8 Trainium2 NeuronCores, data-parallel over the batch dimension (one batch element
per core).

Per-core device kernel layout strategy (all matmul operands bf16):
  - All PE operands are bf16: the xbus streams 2 bytes/cycle/partition, so a
    512-wide bf16 moving tile takes ~213ns vs ~426+ for 4-byte float32r
    ("full-rate" f32r refers to MAC rate, not streaming). Weight DMA halves.
  - LayerNorm in [token, dim] (bn_stats/bn_aggr + fused tensor_scalar-with-
    bf16-cast), then PE-transpose of the bf16 tiles to xnT [dim, token];
    transpose evictions alternate vector/scalar engines. (A DMA-XBAR
    dma_start_transpose variant was ~27 GB/s/queue AND produced
    nondeterministic corruption on hardware — do not revisit.)
  - V lands in [token, feat] (xnT stationary, wv moving) with an all-ones
    column per head so the softmax denominators fall out of the attn @ V
    matmul; V PSUM evictions run on the scalar engine (idle in this phase).
  - Q,K are projected into qT/kT [feat, token]; each head pair's projection is
    slotted 5-matmuls-per-step into the PREVIOUS pair's attention steps.
  - Attention per pair, per j-tile: the two heads' simT[j,i] = k_h^T q_h for
    one i-half go into ONE [128, 1024] fp32 PSUM tile (head0 -> cols 0-511
    from PE rows 0-63, head1 -> cols 512-1023 from rows 64-127). The two
    matmuls target disjoint PE row groups AND disjoint PSUM banks and are
    emitted back-to-back with both becoming ready at the same instant (the
    single exp that frees the tile), so they co-issue concurrently on the
    row-tiled array. exp() on the scalar engine with the 1/8 scale folded
    in writes both heads into one et tile [128, 2048] (blocks h0f0|h1f0|
    h0f1|h1f1); the additive attention bias is ONE vector-engine multiply
    per j-tile by host-precomputed exp(bias)^T in bf16 re-laid to match
    (exp(a+b)=exp(a)exp(b)).
  - PSUM budget (8 banks): 2 sim tiles (4) + head0 av accumulator (2) +
    projection accumulator (2). Head1's AV matmuls are deferred: its et tiles
    are retained in SBUF and the 16-matmul block runs rotated into the NEXT
    pair's stream (overlapping its first exps); the last pair, which has no
    projection work, runs head1's av inline in the projection bank instead.
  - Softmax denominator reciprocals on the DVE: the per-head sum rows are
    DMA-scattered into a [128, 128] layout (head h -> partitions 8h..8h+7)
    so the [16, 1024] reciprocal costs ~1us instead of ~8us, and the Ln/Exp
    reciprocal trick is gone -- it forced ACT_TABLE_LOAD switches (1.3us
    each) mid-stream which stalled the exp pipeline long enough to HAM-
    re-throttle the PE (~27us of half-clock matmuls in the old trace).
    Heads 0-13 are computed early inside the last pair, and the first three
    per-pair normalizations (selection-matrix broadcast matmul + multiply)
    are slotted into the last pair's attention steps, so the tail has ready
    work immediately.
  - Output projection y = outT^T @ w_out runs kt-major in 2-token-tile chunks
    so accumulation starts as soon as the early outT tiles are normalized.

Measured on hardware: ~332us exec per core before the co-issue/reciprocal
rework (8 cores in parallel), rel err ~6.8e-3 vs the fp32 reference
(tolerance 2e-2).
"""

import os

import numpy as np
import ml_dtypes

import concourse.bacc as bacc
import concourse.bass as bass
import concourse.mybir as mybir
import concourse.tile as tile
from concourse.bass_utils import run_bass_kernel_spmd
from concourse.masks import make_identity

B = 8
N = 1024
DIM = 1024
HEADS = 16
DH = 64
INNER = HEADS * DH
P = 128
NT = N // P          # token tiles
KT = DIM // P        # contraction tiles
PAIRS = HEADS // 2   # head pairs (one qT/kT feature tile each)
EPS = 1e-5
SCALE = DH ** -0.5   # 0.125, exact in fp32

F32 = mybir.dt.float32
BF16 = mybir.dt.bfloat16
AF = mybir.ActivationFunctionType

_BUILD_CACHE = {}


def _build(apply_gamma: bool, apply_beta: bool):
    key = (apply_gamma, apply_beta)
    if key in _BUILD_CACHE:
        return _BUILD_CACHE[key]

    nc = bacc.Bacc("TRN2", target_bir_lowering=False, debug=False)

    x_d = nc.dram_tensor("x", [N, DIM], F32, kind="ExternalInput")
    wqk_d = nc.dram_tensor("wqk", [PAIRS, P, KT, 2 * P], BF16, kind="ExternalInput")
    wv_d = nc.dram_tensor("wv", [KT, P, DIM], BF16, kind="ExternalInput")
    wo_d = nc.dram_tensor("wo", [P, KT, DIM], BF16, kind="ExternalInput")
    # exp(bias)^T per head pair, [pair, jt] -> [P(j), 2N] in 512-col blocks
    # (h0 i0:512 | h1 i0:512 | h0 i512:1024 | h1 i512:1024)
    bias_d = nc.dram_tensor("biasT", [PAIRS, NT, P, 2 * N], BF16, kind="ExternalInput")
    sel_d = nc.dram_tensor("sel", [HEADS, KT * P], BF16, kind="ExternalInput")
    gamma_d = beta_d = None
    if apply_gamma:
        gamma_d = nc.dram_tensor("gamma", [DIM], F32, kind="ExternalInput")
    if apply_beta:
        beta_d = nc.dram_tensor("beta", [DIM], F32, kind="ExternalInput")
    y_d = nc.dram_tensor("y", [N, DIM], F32, kind="ExternalOutput")

    with tile.TileContext(nc) as tc:
        from contextlib import ExitStack

        with ExitStack() as ctx:
            consts = ctx.enter_context(tc.tile_pool(name="consts", bufs=1))
            xpool = ctx.enter_context(tc.tile_pool(name="xpool", bufs=3))
            xbpool = ctx.enter_context(tc.tile_pool(name="xbpool", bufs=3))
            stats = ctx.enter_context(tc.tile_pool(name="stats", bufs=4))
            bigp = ctx.enter_context(tc.tile_pool(name="bigp", bufs=1))
            vpool = ctx.enter_context(tc.tile_pool(name="vpool", bufs=NT))
            wstream = ctx.enter_context(tc.tile_pool(name="wstream", bufs=3))
            qkpool = ctx.enter_context(tc.tile_pool(name="qkpool", bufs=4))
            epool = ctx.enter_context(tc.tile_pool(name="epool", bufs=NT + 2))
            bpool = ctx.enter_context(tc.tile_pool(name="bpool", bufs=3))
            opool = ctx.enter_context(tc.tile_pool(name="opool", bufs=KT))
            wvpool = ctx.enter_context(tc.tile_pool(name="wvpool", bufs=KT))

            eps_t = consts.tile([P, 1], F32, name="eps_t")
            nc.vector.memset(eps_t, EPS)
            ident_bf = consts.tile([P, P], BF16, name="ident_bf")
            make_identity(nc, ident_bf)
            S = consts.tile([HEADS, KT * P], BF16, name="S")
            nc.sync.dma_start(out=S, in_=sel_d[:, :])
            # Denominator sums in a [128, 128] layout: head h's 1024-token
            # sum row lives on partitions 8h..8h+7 (128 tokens each), so the
            # DVE reciprocal streams 128 elements per partition, not 1024.
            sums_rs = consts.tile([P, P], BF16, name="sums_rs")
            recip_rs = consts.tile([P, P], BF16, name="recip_rs")
            recip = consts.tile([HEADS, N], BF16, name="recip")
            nc.vector.memset(recip, 1.0)

            gamma_t = beta_t = None
            if apply_gamma:
                gamma_t = consts.tile([P, DIM], F32, name="gamma_t")
                g_ap = gamma_d[:]
                nc.sync.dma_start(
                    out=gamma_t,
                    in_=bass.AP(
                        tensor=g_ap.tensor, offset=g_ap.offset, ap=[[0, P]] + list(g_ap.ap)
                    ),
                )
            if apply_beta:
                beta_t = consts.tile([P, DIM], F32, name="beta_t")
                b_ap = beta_d[:]
                nc.sync.dma_start(
                    out=beta_t,
                    in_=bass.AP(
                        tensor=b_ap.tensor, offset=b_ap.offset, ap=[[0, P]] + list(b_ap.ap)
                    ),
                )

            xnT = bigp.tile([P, KT, N], BF16, name="xnT", tag="big")

            vts = []
            for jt in range(NT):
                vt = vpool.tile([P, HEADS * (DH + 1)], BF16, name=f"v{jt}", tag="v")
                vv = vt.rearrange("p (h c) -> p h c", c=DH + 1)
                nc.vector.memset(vv[:, :, DH : DH + 1], 1.0)
                vts.append((vt, vv))

            # ================= Phase A: LayerNorm + DMA transpose + V =======
            def emit_ln(it):
                xt = xpool.tile([P, DIM], F32, name=f"x{it}", tag="x")
                nc.sync.dma_start(out=xt, in_=x_d[it * P : (it + 1) * P, :])
                st = stats.tile([P, 2, 6], F32, name=f"st{it}", tag="st")
                nc.vector.bn_stats(out=st[:, 0], in_=xt[:, 0:512])
                nc.vector.bn_stats(out=st[:, 1], in_=xt[:, 512:1024])
                mv = stats.tile([P, 2], F32, name=f"mv{it}", tag="mv")
                nc.vector.bn_aggr(out=mv, in_=st)
                std = stats.tile([P, 1], F32, name=f"sd{it}", tag="sd")
                nc.scalar.activation(out=std, in_=mv[:, 1:2], func=AF.Sqrt, bias=eps_t)
                rstd = stats.tile([P, 1], F32, name=f"rs{it}", tag="rs")
                nc.vector.reciprocal(out=rstd, in_=std)
                xb = xbpool.tile([P, DIM], BF16, name=f"xb{it}", tag="xb")
                if gamma_t is None and beta_t is None:
                    nc.vector.tensor_scalar(
                        out=xb,
                        in0=xt,
                        scalar1=mv[:, 0:1],
                        scalar2=rstd,
                        op0=mybir.AluOpType.subtract,
                        op1=mybir.AluOpType.mult,
                    )
                else:
                    nc.vector.tensor_scalar(
                        out=xt,
                        in0=xt,
                        scalar1=mv[:, 0:1],
                        scalar2=rstd,
                        op0=mybir.AluOpType.subtract,
                        op1=mybir.AluOpType.mult,
                    )
                    if gamma_t is not None:
                        nc.vector.tensor_mul(xt, xt, gamma_t)
                    if beta_t is not None:
                        nc.vector.tensor_add(xt, xt, beta_t)
                    nc.vector.tensor_copy(xb, xt)
                # PE transpose each [128,128] bf16 block into xnT (the
                # DMA-XBAR transpose corrupted results on hardware — its
                # completion sync raced the consuming matmuls — and at
                # ~27 GB/s/queue it was also slower than the PE).
                for kt in range(KT):
                    pt = ptpool.tile([P, P], BF16, name=f"tp{it}_{kt}", tag="tp")
                    nc.tensor.transpose(
                        pt, xb[:, kt * P : (kt + 1) * P], ident_bf
                    )
                    if kt % 2 == 0:
                        nc.vector.tensor_copy(
                            xnT[:, kt, it * P : (it + 1) * P], pt
                        )
                    else:
                        nc.scalar.copy(
                            out=xnT[:, kt, it * P : (it + 1) * P], in_=pt
                        )

            wvts = []
            with tc.tile_pool(name="psA", bufs=2, space="PSUM") as psA, \
                 tc.tile_pool(name="ptp", bufs=4, space="PSUM") as ptpool:

                def emit_v_group(g):
                    psv = [
                        psA.tile([P, DIM], F32, name=f"psv{g}_{j}", tag="psv")
                        for j in range(2)
                    ]
                    for kt in range(KT):
                        if g == 0:
                            wvt = wvpool.tile(
                                [P, DIM], BF16, name=f"wv{kt}", tag="wv"
                            )
                            nc.sync.dma_start(out=wvt, in_=wv_d[kt])
                            wvts.append(wvt)
                        wvt = wvts[kt]
                        for j in range(2):
                            jt = 2 * g + j
                            for hf in range(2):
                                sl = slice(hf * 512, hf * 512 + 512)
                                nc.tensor.matmul(
                                    psv[j][:, sl],
                                    lhsT=xnT[:, kt, jt * P : (jt + 1) * P],
                                    rhs=wvt[:, sl],
                                    start=(kt == 0),
                                    stop=(kt == KT - 1),
                                )
                    # Evictions on the scalar engine (idle in this phase).
                    for j in range(2):
                        jt = 2 * g + j
                        vv = vts[jt][1]
                        for hf in range(2):
                            nc.scalar.copy(
                                out=vv[:, hf * 8 : hf * 8 + 8, 0:DH],
                                in_=psv[j][
                                    :, hf * 512 : hf * 512 + 512
                                ].rearrange("p (h c) -> p h c", c=DH),
                            )

                for g in range(NT // 2):
                    emit_ln(2 * g)
                    emit_ln(2 * g + 1)
                    emit_v_group(g)

            # ================= Phase B: QK proj + attention =================
            wqs, qTs, kTs = [], [], []
            wq0 = wstream.tile([P, KT, 2 * P], BF16, name="wqk0", tag="w")
            nc.sync.dma_start(out=wq0, in_=wqk_d[0])
            wqs.append(wq0)
            qTs.append(qkpool.tile([P, N], BF16, name="qT0", tag="qk"))
            kTs.append(qkpool.tile([P, N], BF16, name="kT0", tag="qk"))

            with ExitStack() as actx:
                sim_pool = actx.enter_context(
                    tc.tile_pool(name="simp", bufs=2, space="PSUM")
                )
                av_pool = actx.enter_context(
                    tc.tile_pool(name="avp", bufs=1, space="PSUM")
                )
                proj_pool = actx.enter_context(
                    tc.tile_pool(name="projp", bufs=1, space="PSUM")
                )

                def proj_steps(pn, which):
                    """Generator: 16 matmuls (kt-outer, hf-inner) accumulating
                    pair pn's q (which=0) or k (which=1) projection, then
                    evicts to SBUF. Yields after each matmul."""
                    ps = proj_pool.tile(
                        [P, N], F32, name=f"ps{'qk'[which]}{pn}", tag="proj"
                    )
                    w0 = which * P
                    for kt in range(KT):
                        for hf in range(2):
                            sl = slice(hf * 512, hf * 512 + 512)
                            nc.tensor.matmul(
                                ps[:, sl],
                                lhsT=wqs[pn][:, kt, w0 : w0 + P],
                                rhs=xnT[:, kt, sl],
                                start=(kt == 0),
                                stop=(kt == KT - 1),
                            )
                            yield
                    nc.vector.tensor_copy((qTs, kTs)[which][pn], ps)
                    while True:
                        yield

                # Pair 0's projections run standalone (prologue).
                for which in range(2):
                    g = proj_steps(0, which)
                    for _ in range(17):
                        next(g)

                outTs = []

                # et block offset for (head-in-pair hh, i-half hf)
                def eoff(hh, hf):
                    return hf * N + hh * 512

                def emit_av(avt, pp, ets_, hh, jt):
                    h = 2 * pp + hh
                    for hf in range(2):
                        nc.tensor.matmul(
                            avt[:, hf * 512 : hf * 512 + 512],
                            lhsT=vts[jt][0][:, h * (DH + 1) : (h + 1) * (DH + 1)],
                            rhs=ets_[jt][:, eoff(hh, hf) : eoff(hh, hf) + 512],
                            start=(jt == 0),
                            stop=(jt == NT - 1),
                        )

                def emit_evict(avt, pp, hh):
                    h = 2 * pp + hh
                    hs = slice(hh * DH, (hh + 1) * DH)
                    # Evict via SBUF staging (DMA cannot read PSUM; DVE
                    # cannot shift partitions — stage on matching partitions,
                    # then DMA to the head's row block in outT, and scatter
                    # the denominator row into sums_rs [8h..8h+8, 128].
                    avs = xpool.tile([DH + 1, N], BF16, name=f"avs{h}", tag="avs")
                    nc.vector.tensor_copy(avs, avt)
                    nc.sync.dma_start(out=outTs[pp][hs, :], in_=avs[0:DH, :])
                    nc.sync.dma_start(
                        out=sums_rs[8 * h : 8 * h + 8, :], in_=avs[DH : DH + 1, :]
                    )

                def emit_norm_kt(kt, pool):
                    # Normalize pair kt's outT rows: broadcast recip rows
                    # [2kt, 2kt+2) over the 128 feature rows (S columns for
                    # other heads are zero) and multiply.
                    rs = pool.tile([P, N], F32, name=f"rs{kt}", tag="sim")
                    for hf in range(2):
                        sl = slice(hf * 512, hf * 512 + 512)
                        nc.tensor.matmul(
                            rs[:, sl],
                            lhsT=S[:, kt * P : (kt + 1) * P],
                            rhs=recip[:, sl],
                            start=True,
                            stop=True,
                        )
                    nc.vector.tensor_mul(outTs[kt], outTs[kt], rs)

                def emit_av1_block(pp, ets_):
                    # Head1's AV for pair pp, run from the retained et tiles
                    # (the av bank is free after head0's eviction).
                    av1 = av_pool.tile([DH + 1, N], F32, name=f"av{2*pp+1}", tag="av")
                    for jt in range(NT):
                        emit_av(av1, pp, ets_, 1, jt)
                    emit_evict(av1, pp, 1)

                def emit_recip(p0, p1):
                    # DVE reciprocal of the denominator sums for heads
                    # [p0, p1) in the [128, 128] layout, then gather back to
                    # the [16, 1024] recip tile the norm matmuls read.
                    with nc.allow_low_precision(reason="bf16 softmax denominators"):
                        nc.vector.reciprocal(
                            out=recip_rs[8 * p0 : 8 * p1, :],
                            in_=sums_rs[8 * p0 : 8 * p1, :],
                        )
                    nc.sync.dma_start(
                        out=recip[p0:p1, :], in_=recip_rs[8 * p0 : 8 * p1, :]
                    )

                prev_ets = None
                for p in range(PAIRS):
                    qT, kTt = qTs[p], kTs[p]
                    if p + 1 < PAIRS:
                        wq = wstream.tile(
                            [P, KT, 2 * P], BF16, name=f"wqk{p+1}", tag="w"
                        )
                        nc.sync.dma_start(out=wq, in_=wqk_d[p + 1])
                        wqs.append(wq)
                        qTs.append(
                            qkpool.tile([P, N], BF16, name=f"qT{p+1}", tag="qk")
                        )
                        kTs.append(
                            qkpool.tile([P, N], BF16, name=f"kT{p+1}", tag="qk")
                        )

                    ot = opool.tile([P, N], BF16, name=f"outT{p}", tag="outT")
                    outTs.append(ot)
                    ets = []
                    av0 = None

                    if p + 1 < PAIRS:
                        projq = proj_steps(p + 1, 0)
                        projk = proj_steps(p + 1, 1)
                    else:
                        projq = projk = iter(())
                    pstep = [0]

                    def proj_step(k):
                        # Steps 0-16 drive the q projection (16 matmuls + its
                        # eviction), 17-33 the k projection.
                        for _ in range(k):
                            if pstep[0] < 17:
                                next(projq, None)
                            elif pstep[0] < 34:
                                next(projk, None)
                            pstep[0] += 1

                    av1l = None  # last pair only: av1 inline in the proj bank
                    for jt in range(NT):
                        bt = bpool.tile([P, 2 * N], BF16, name=f"b{p}_{jt}", tag="bias")
                        nc.sync.dma_start(out=bt, in_=bias_d[p, jt])
                        # Two i-halves; each half's two heads go to one PSUM
                        # tile (head0 -> cols 0-511 from PE rows 0-63, head1
                        # -> cols 512-1023 from rows 64-127). The matmuls are
                        # emitted adjacent, write disjoint PSUM banks, and
                        # become ready together (the one exp below frees the
                        # tile), so they co-issue on the row-tiled PE.
                        et = epool.tile([P, 2 * N], BF16, name=f"e{p}_{jt}", tag="exp")
                        for hf in range(2):
                            st = sim_pool.tile(
                                [P, N], F32, name=f"sim{p}_{jt}_{hf}", tag="sim"
                            )
                            for hh in range(2):
                                hs = slice(hh * DH, (hh + 1) * DH)
                                nc.tensor.matmul(
                                    st[:, hh * 512 : hh * 512 + 512],
                                    lhsT=kTt[hs, jt * P : (jt + 1) * P],
                                    rhs=qT[hs, hf * 512 : hf * 512 + 512],
                                    start=True,
                                    stop=True,
                                )
                            nc.scalar.activation(
                                out=et[:, hf * N : (hf + 1) * N],
                                in_=st,
                                func=AF.Exp,
                                scale=SCALE,
                            )
                        nc.vector.tensor_mul(et, et, bt)
                        ets.append(et)
                        if jt == 1 and prev_ets is not None:
                            # Rotated: the previous pair's head1 AV block runs
                            # here, overlapping this pair's first exps.
                            emit_av1_block(p - 1, prev_ets)
                            if p == PAIRS - 1:
                                # Pairs 0-6 denominators are final: compute
                                # their reciprocals now (heads 0-13) so the
                                # normalization needs no serial chain at the
                                # tail.
                                emit_recip(0, 14)
                        if p == PAIRS - 1 and jt in (3, 5, 7):
                            # Slot the first three normalizations into the
                            # last pair's stream (it has no projection work).
                            emit_norm_kt((jt - 3) // 2, sim_pool)
                        if jt > 0:
                            if av0 is None:
                                av0 = av_pool.tile(
                                    [DH + 1, N], F32, name=f"av{2*p}", tag="av"
                                )
                            emit_av(av0, p, ets, 0, jt - 1)
                            if p == PAIRS - 1:
                                # No projections in the last pair: its av1
                                # accumulates inline in the free proj bank.
                                if av1l is None:
                                    av1l = proj_pool.tile(
                                        [DH + 1, N], F32, name="av15", tag="proj"
                                    )
                                emit_av(av1l, p, ets, 1, jt - 1)
                        proj_step(5 if jt < 6 else 4)

                    emit_av(av0, p, ets, 0, NT - 1)
                    proj_step(4)  # drain the projection evictions
                    emit_evict(av0, p, 0)
                    if p == PAIRS - 1:
                        emit_av(av1l, p, ets, 1, NT - 1)
                        emit_evict(av1l, p, 1)
                    else:
                        prev_ets = ets

            # ================= Phase C: normalize + y = outT^T @ w_out ======
            wo_t = bigp.tile([P, KT, DIM], BF16, name="wo_t", tag="big")
            for kt in range(KT):
                nc.sync.dma_start(out=wo_t[:, kt, :], in_=wo_d[:, kt, :])

            with ExitStack() as tctx:
                rs_pool = tctx.enter_context(
                    tc.tile_pool(name="rsp", bufs=2, space="PSUM")
                )
                psy_pool = tctx.enter_context(
                    tc.tile_pool(name="psyp", bufs=4, space="PSUM")
                )

                # Finish the reciprocals (heads 12-15 — DVE start partitions
                # must be quadrant-aligned, so heads 12-13 are recomputed)
                # and the remaining normalizations; kts 0-2 were normalized
                # inside pair 7, so the output projection has ready operands
                # immediately and the PE never idles into a HAM re-throttle.
                emit_recip(12, 16)
                for kt in range(3, KT):
                    emit_norm_kt(kt, rs_pool)

                # kt-major output projection in chunks of 2 token tiles: the
                # first chunk's kt accumulation starts as soon as outT[kt] is
                # normalized, keeping the PE dense through the tail.
                for c in range(NT // 2):
                    psys = {}
                    for j in range(2):
                        it = 2 * c + j
                        for hf in range(2):
                            psys[(j, hf)] = psy_pool.tile(
                                [P, 512], F32, name=f"psy{it}_{hf}", tag="psy"
                            )
                    for kt in range(KT):
                        for j in range(2):
                            it = 2 * c + j
                            for hf in range(2):
                                nc.tensor.matmul(
                                    psys[(j, hf)],
                                    lhsT=outTs[kt][:, it * P : (it + 1) * P],
                                    rhs=wo_t[:, kt, hf * 512 : hf * 512 + 512],
                                    start=(kt == 0),
                                    stop=(kt == KT - 1),
                                )
                    for j in range(2):
                        it = 2 * c + j
                        yst = xpool.tile([P, DIM], F32, name=f"y{it}", tag="x")
                        nc.vector.tensor_copy(yst[:, 0:512], psys[(j, 0)])
                        nc.scalar.copy(out=yst[:, 512:1024], in_=psys[(j, 1)])
                        nc.sync.dma_start(
                            out=y_d[it * P : (it + 1) * P, :], in_=yst
                        )

    nc.compile()
    _BUILD_CACHE[key] = nc
    return nc


def _host_prep(ln_gamma, ln_beta, w_qkv, w_out, attn_bias):
    """Re-layout weights/bias for the device kernel (pure host-side reshapes)."""
    w_qkv = np.asarray(w_qkv, np.float32)
    w_out = np.asarray(w_out, np.float32)
    attn_bias = np.asarray(attn_bias, np.float32)

    wq_r = w_qkv[:, :INNER].reshape(KT, P, PAIRS, P).transpose(2, 1, 0, 3)
    wk_r = w_qkv[:, INNER : 2 * INNER].reshape(KT, P, PAIRS, P).transpose(2, 1, 0, 3)
    wqk = np.ascontiguousarray(
        np.concatenate([wq_r, wk_r], axis=3).astype(ml_dtypes.bfloat16)
    )
    wv = np.ascontiguousarray(
        w_qkv[:, 2 * INNER :].reshape(KT, P, DIM).astype(ml_dtypes.bfloat16)
    )
    wo = np.ascontiguousarray(
        w_out.reshape(KT, P, DIM).transpose(1, 0, 2).astype(ml_dtypes.bfloat16)
    )
    # exp(bias), transposed per head to [j, i], paired and blocked to match
    # the device et layout: [pair, jt, P, (hf, hh, 512)].
    ebT = (
        np.exp(attn_bias[0].astype(np.float64))
        .astype(np.float32)
        .transpose(0, 2, 1)
        .astype(ml_dtypes.bfloat16)
    )  # [HEADS, N(j), N(i)]
    biasT = np.ascontiguousarray(
        ebT.reshape(PAIRS, 2, NT, P, 2, 512)
        .transpose(0, 2, 3, 4, 1, 5)
        .reshape(PAIRS, NT, P, 2 * N)
    )
    sel = np.zeros((HEADS, KT * P), dtype=ml_dtypes.bfloat16)
    for h in range(HEADS):
        c0 = (h // 2) * P + (h % 2) * DH
        sel[h, c0 : c0 + DH] = 1.0
    in_map = {"wqk": wqk, "wv": wv, "wo": wo, "biasT": biasT, "sel": sel}

    gamma = np.asarray(ln_gamma, np.float32)
    beta = np.asarray(ln_beta, np.float32)
    apply_gamma = not np.all(gamma == 1.0)
    apply_beta = bool(np.any(beta != 0.0))
    if apply_gamma:
        in_map["gamma"] = gamma
    if apply_beta:
        in_map["beta"] = beta
    return in_map, apply_gamma, apply_beta


def kernel(x, ln_gamma, ln_beta, w_qkv, w_out, attn_bias):
    x = np.asarray(x, np.float32)
    in_map, apply_gamma, apply_beta = _host_prep(
        ln_gamma, ln_beta, w_qkv, w_out, attn_bias
    )
    nc = _build(apply_gamma, apply_beta)
    in_maps = [dict(in_map, x=np.ascontiguousarray(x[b])) for b in range(B)]
    res = run_bass_kernel_spmd(
        nc,
        in_maps,
        list(range(B)),
        trace=bool(int(os.environ.get("BA_TRACE", "0"))),
        tmpdir=os.environ.get("BA_TRACE_DIR") or None,
    )
    out = np.stack([res.results[i]["y"] for i in range(B)], axis=0)
    if bool(int(os.environ.get("BA_TRACE", "0"))):
        kernel.last_exec_time_ns = res.exec_time_ns
        kernel.last_mean_exec_time_ns = res.mean_exec_time_ns
    return out
